# revision 46
# baseline (speedup 1.0000x reference)
"""Dynamic per-sample CNN (nn_ConvFunc) Trainium2 Bass kernel.

Reference computation (per sample b):
  cnn_inp = proj_w @ cat(lhs, rhs) + proj_b          # 1x1 conv, [128, 32, 32]
  out     = conv3x3(cnn_inp, W_b) + bias_b           # W_b, bias_b unpacked from question_rep[b]

Sharding: pure data parallel, 8 samples per NeuronCore (batch 64 / 8 cores).

Per-core device kernel, per sample:
  - proj: per 512-col half of the 32x32 pixel space, 2 accumulating bf16
    matmuls (lhs-channels, rhs-channels) into one PSUM bank; DVE evicts PSUM
    (+proj_b per-partition bias) into the interior of a zero-bordered
    [128,34,34] padded SBUF tile;
  - conv: per half, 9 taps of accumulating bf16 matmuls; rhs = shifted 16x32
    window of the padded tile (strided AP); DVE evicts PSUM (+cnn bias) to
    bf16 SBUF, one store DMA per sample (split for the last sample so the
    final store issues early).

All matmul operands are bf16 (rounded on host): the PE streams 1 col/cycle
either way, but bf16 halves HBM traffic and triggers FWL so LDWEIGHTS hides
behind the previous matmul.

Scheduling notes (hard-won):
- The Tile scheduler is a readiness+priority simulator with an optimistic
  DMA model; left alone it reorders the Tensor queue and serializes the PE
  behind late DMAs. tick() stamps (tile_set_cur_wait) force the schedule.
- The PE must have NO idle gaps >~0.5us during the ramp or the HAM clock
  gate re-throttles to 1.2GHz (costs ~6us); dummy matmuls plug every
  DMA-jitter window.
- Tile has only 8 shared HWDGE DMA sem lanes: a 9th in-flight queue op
  stalls its ISSUING ENGINE until an older DMA completes. Loads are
  software-pipelined ~3 samples ahead, never all-upfront.
- Evictions (~700ns each, 4/sample, PSUM reads pinned to 1 elem/cycle) are
  split ACT/DVE so pairs run in parallel on different banks; store queue
  ops must NOT ride the scalar engine (they'd block ACT evictions).
- The last sample's second half accumulates into separate PSUM banks per
  quarter so the final evict+store chain is short and parallel.
"""

import numpy as np
import ml_dtypes

import concourse.bass as bass
import concourse.mybir as mybir
from concourse import bacc
from concourse.tile import TileContext
from concourse.bass_utils import run_bass_kernel_spmd

# Problem shapes (hardcoded per contract)
B = 64
DIM = 128
H = W = 32
K = 3
KK = K * K
HW = H * W             # 1024
WDIM = DIM * DIM * KK  # 147456
NCORES = 8
SPC = B // NCORES      # samples per core
HP, WP = H + 2, W + 2  # padded 34x34
HALF = HW // 2         # 512 columns per PSUM bank
QUART = HALF // 2
HROWS = H // 2         # 16 output rows per half
NXP = 4                # persistent padded tiles (cycled s % NXP)
N_WARM = 26            # dummy warmup matmuls (N=128 each)

FP = mybir.dt.float32
BF = mybir.dt.bfloat16
BF_NP = ml_dtypes.bfloat16

_BUILT = {}


def build_nc():
    # enable_partition_id=False: the kernel never branches on core id (inputs
    # are pre-sharded host-side), and the partition-id machinery costs ~1.7us
    # of per-engine TENSOR_LOADs in the program preamble.
    nc = bacc.Bacc("TRN2", target_bir_lowering=False, debug=False,
                   num_devices=NCORES, enable_partition_id=False)

    # xc packs [xl_h0 | xr_h0 | xl_h1 | xr_h1] per sample (one load DMA)
    xc = nc.declare_dram_parameter("xc", [SPC, DIM, 4, HALF], BF, isOutput=False)
    qw = nc.declare_dram_parameter("qw", [SPC, DIM, KK * DIM], BF, isOutput=False)
    # pwx packs [proj weights (256) | sample-0 xl_h0 (512)] so the entire
    # critical chain of the first real matmul lands with ONE DMA completion
    # (per-op completion latency ~2.3us dominates the ramp)
    pwx = nc.declare_dram_parameter("pwx", [DIM, 2 * DIM + HALF], BF,
                                    isOutput=False)
    bia = nc.declare_dram_parameter("bia", [DIM, SPC + 1], FP, isOutput=False)
    out = nc.declare_dram_parameter("out", [SPC, DIM, HW], BF, isOutput=True)

    with TileContext(nc) as tc:
        with (
            tc.tile_pool(name="const", bufs=1) as cpool,
            tc.tile_pool(name="wpool", bufs=6) as wpool,
            tc.tile_pool(name="xpool", bufs=6) as xpool,
            tc.tile_pool(name="opool", bufs=6) as opool,
            tc.tile_pool(name="pp_pool", bufs=4, space="PSUM") as pp_pool,
            tc.tile_pool(name="pc_pool", bufs=4, space="PSUM") as pc_pool,
        ):
            # --- warmup: short dummy matmuls keep the PE busy through the
            # DMA ramp so the HAM clock gate lifts before real work
            # memsets ride GpSimd: it exits the framework preamble earliest
            # and is otherwise idle, so the first warmup matmul issues sooner
            dummy = cpool.tile([DIM, DIM], BF)
            nc.gpsimd.memset(dummy[:], 0.0)
            pdt = pc_pool.tile([DIM, HALF], FP, tag="pc")

            def warm(n):
                for _ in range(n):
                    nc.tensor.matmul(pdt[:, 0:DIM], lhsT=dummy[:], rhs=dummy[:],
                                     start=True, stop=True)

            warm(N_WARM)

            # --- constants: pwx (proj weights + sample-0 xl_h0) FIRST on the
            # sync ring — one completion unblocks the first real matmul pair;
            # biases on scalar (needed only by the first eviction, later)
            pwx_sb = cpool.tile([DIM, 2 * DIM + HALF], BF)
            nc.sync.dma_start(out=pwx_sb[:], in_=pwx[:])
            bia_sb = cpool.tile([DIM, SPC + 1], FP)
            nc.scalar.dma_start(out=bia_sb[:], in_=bia[:])
            pw0 = pwx_sb[:, 0:DIM]
            pw1 = pwx_sb[:, DIM:2 * DIM]
            xl0_ap = pwx_sb[:, 2 * DIM:2 * DIM + HALF]

            def qb_ap(s):
                return bia_sb[:, s:s + 1]

            pb_ap = bia_sb[:, SPC:SPC + 1]

            # --- persistent padded tiles: borders zeroed once, interiors
            # rewritten per sample
            # xp borders on DVE (85-97ns each there vs 124-312 on GpSimd;
            # DVE is idle until the first eviction ~11us)
            xp_tiles = []
            for i in range(NXP):
                xp = cpool.tile([DIM, HP, WP], BF, tag=f"xp{i}")
                nc.vector.memset(xp[:, 0:1, :], 0.0)
                nc.vector.memset(xp[:, HP - 1:HP, :], 0.0)
                nc.vector.memset(xp[:, 1:HP - 1, 0:1], 0.0)
                nc.vector.memset(xp[:, 1:HP - 1, WP - 1:WP], 0.0)
                xp_tiles.append(xp)

            # ALL loads ride the sync ring in strict consumption order: the
            # ramp is bandwidth-bound (~150GB/s until ~11.5us, ~300+ after),
            # so sync-ring FIFO order IS the byte priority. The scalar ring
            # carries only bia and the per-sample stores, keeping the ACT
            # engine's instruction stream free of load queue ops.
            def load_x(s):
                xc_sb = xpool.tile([DIM, 4, HALF], BF, tag="xc")
                if s == 0:
                    # sample-0 xl_h0 rides inside pwx; xr_h0 then h1 (NOT
                    # finer: an 8th upfront queue op trips the 8-lane limit
                    # and stalls the whole ring's issue cadence)
                    nc.sync.dma_start(out=xc_sb[:, 1:2], in_=xc[s, :, 1:2])
                    nc.sync.dma_start(out=xc_sb[:, 2:4], in_=xc[s, :, 2:4])
                elif s <= 3:
                    # samples 1-3 sit on the ramp's byte wall: half-grain
                    # loads let proj(s)-h0 start ~0.85us before h1 lands
                    nc.sync.dma_start(out=xc_sb[:, 0:2], in_=xc[s, :, 0:2])
                    nc.sync.dma_start(out=xc_sb[:, 2:4], in_=xc[s, :, 2:4])
                else:
                    nc.sync.dma_start(out=xc_sb[:], in_=xc[s])
                return xc_sb

            def load_w(s):
                w_sb = wpool.tile([DIM, KK, DIM], BF, tag="w")
                if s == 0:
                    # tap-chunks so conv(0)'s first taps start on chunk 1
                    # (Tile's dependency tracking is range-precise)
                    for t0, t1 in ((0, 3), (3, KK)):
                        nc.sync.dma_start(out=w_sb[:, t0:t1, :],
                                          in_=qw[s, :, t0 * DIM:t1 * DIM])
                else:
                    # latency-tolerant qw loads ride the idle GpSimd SWDGE
                    # ring: frees the sync ring's issue cadence + sem lanes
                    # for the xc stream during the ramp
                    nc.gpsimd.dma_start(out=w_sb[:], in_=qw[s])
                return w_sb

            # tick(): advance the Tile scheduler's logical clock. The
            # scheduler is a readiness+priority simulator whose DMA cost
            # model is optimistic — left alone it reorders the Tensor queue
            # (e.g. hoisting proj(s+1) above conv(s), which then serializes
            # the PE behind a late xc DMA and resets the HAM window).
            # Stamping each pipeline unit with an increasing wait_until makes
            # the compile-time order match THIS emission order; no runtime
            # waits are emitted (sems remain dependency-only).
            _clk = [0.0]

            def tick():
                _clk[0] += 0.05
                tc.tile_set_cur_wait(_clk[0])

            def proj(s, xc_sb):
                xp = xp_tiles[s % NXP]
                for h in range(2):
                    tick()
                    ppt = pp_pool.tile([DIM, HALF], FP, tag="pp")
                    xl_rhs = xl0_ap if (s == 0 and h == 0) else xc_sb[:, 2 * h]
                    nc.tensor.matmul(ppt[:], lhsT=pw0, rhs=xl_rhs,
                                     start=True, stop=False)
                    if s == 0 and h == 0:
                        # plug the xl->xr DMA-completion gap: PE-idle gaps
                        # during the ramp reset the HAM activity window
                        # (dummies write a different PSUM bank, which leaves
                        # ppt's accumulation group intact)
                        tick()
                        warm(4)
                        tick()
                    nc.tensor.matmul(ppt[:], lhsT=pw1, rhs=xc_sb[:, 2 * h + 1],
                                     start=False, stop=True)
                    # evictions are the serialization point (~750ns each, 4 per
                    # sample, PSUM reads pinned to 1 elem/cyc): split them over
                    # ACT (h0) and DVE (h1) so the two proj evicts run in
                    # parallel on different banks and conv(s)'s first taps
                    # (which need BOTH) unblock sooner
                    tick()
                    dst = xp[:, 1 + HROWS * h:1 + HROWS * (h + 1), 1:1 + W]
                    src = ppt[:].rearrange("p (a b) -> p a b", b=W)
                    if h == 0:
                        nc.scalar.activation(
                            dst, src, mybir.ActivationFunctionType.Identity,
                            bias=pb_ap)
                    else:
                        nc.vector.tensor_scalar_add(dst, src, pb_ap)
                    if s == 0 and h == 0:
                        # plug the xc0-h1 DMA wait (xl_h1 lands ~1.2us after
                        # xr at ramp bandwidth) so the HAM window survives
                        tick()
                        warm(12)
                return xp

            def conv(s, xp, w_sb):
                tick()
                o_sb = opool.tile([DIM, HW], BF, tag="o")
                last = s == SPC - 1
                if last:
                    # LAST sample, h1: accumulate the two quarters into TWO
                    # separate PSUM banks so quarter A's eviction + store
                    # overlap quarter B's taps, and the two evictions run on
                    # different banks (ACT || DVE) instead of serializing on
                    # one bank's single read port. Shrinks the kernel tail.
                    pct0 = pc_pool.tile([DIM, HALF], FP, tag="pc")
                    for t in range(KK):
                        kh, kw = divmod(t, K)
                        nc.tensor.matmul(
                            pct0[:],
                            lhsT=w_sb[:, t, :],
                            rhs=xp[:, kh:HROWS + kh, kw:kw + W],
                            start=(t == 0), stop=(t == KK - 1))
                    nc.vector.tensor_scalar_add(
                        o_sb[:, 0:HALF], pct0[:], qb_ap(s))
                    nc.sync.dma_start(out=out[s, :, 0:HALF],
                                      in_=o_sb[:, 0:HALF])
                    qrows = HROWS // 2  # 8 output rows per quarter
                    pcq = []
                    for q in range(2):
                        tick()
                        pq = pc_pool.tile([DIM, HALF], FP, tag="pc")
                        base = HROWS + qrows * q
                        for t in range(KK):
                            kh, kw = divmod(t, K)
                            nc.tensor.matmul(
                                pq[:, 0:QUART],
                                lhsT=w_sb[:, t, :],
                                rhs=xp[:, base + kh:base + qrows + kh,
                                       kw:kw + W],
                                start=(t == 0), stop=(t == KK - 1))
                        pcq.append(pq)
                        if q == 0:
                            # qA: DVE evict + sync store (overlaps qB's taps)
                            nc.vector.tensor_scalar_add(
                                o_sb[:, HALF:HALF + QUART], pq[:, 0:QUART],
                                qb_ap(s))
                            nc.sync.dma_start(
                                out=out[s, :, HALF:HALF + QUART],
                                in_=o_sb[:, HALF:HALF + QUART])
                    # final quarter: ACT evict (1.2GHz, fastest PSUM read)
                    # + store on the scalar ring, which has been empty since
                    # bia — no lane-reuse wait, fastest possible issue
                    nc.scalar.activation(
                        o_sb[:, HALF + QUART:HW], pcq[1][:, 0:QUART],
                        mybir.ActivationFunctionType.Identity,
                        bias=qb_ap(s))
                    nc.scalar.dma_start(out=out[s, :, HALF + QUART:HW],
                                        in_=o_sb[:, HALF + QUART:HW])
                    return
                for h in range(2):
                    tick()
                    pct = pc_pool.tile([DIM, HALF], FP, tag="pc")
                    for t in range(KK):
                        if s == 0 and h == 0 and t in (3, 6):
                            # absorb qw0 tap-chunk DMA jitter without a
                            # PE-idle gap (HAM window protection)
                            tick()
                            warm(2)
                            tick()
                        kh, kw = divmod(t, K)
                        nc.tensor.matmul(
                            pct[:],
                            lhsT=w_sb[:, t, :],
                            rhs=xp[:, HROWS * h + kh:HROWS * (h + 1) + kh,
                                   kw:kw + W],
                            start=(t == 0), stop=(t == KK - 1))
                    # conv evicts: h0 on DVE, h1 on ACT (proj uses the
                    # opposite pairing, balancing both engines)
                    dst = o_sb[:, h * HALF:(h + 1) * HALF]
                    if h == 0:
                        nc.vector.tensor_scalar_add(dst, pct[:], qb_ap(s))
                    else:
                        nc.scalar.activation(
                            dst, pct[:],
                            mybir.ActivationFunctionType.Identity,
                            bias=qb_ap(s))
                # store on the SYNC engine: a store queue op on the scalar
                # engine can block later ACT evictions for microseconds (it
                # waits on its DMA sem-lane's previous user + both evicts),
                # which starves conv(s+1) and resets the HAM window. The
                # sync engine has finished issuing all prefetched loads by
                # the time stores appear.
                nc.sync.dma_start(out=out[s], in_=o_sb[:])

            # Prefetch depth is bounded by Tile's 8 shared HWDGE sem lanes:
            # the 9th+ in-flight DMA queue op STALLS its issuing engine
            # waiting for an older DMA's completion. So only samples 0-1
            # load upfront (7 ops, unstamped = issue immediately); later
            # samples are emitted in-loop ~3 samples ahead — their stamps
            # give them issue slots well before consumption with only ~3
            # ops in flight.
            xcs = {0: load_x(0)}
            ws = {0: load_w(0)}
            xcs[1] = load_x(1)
            ws[1] = load_w(1)
            # Forced PE schedule (tick-stamped): proj0 -> conv0 (conv0's
            # 3.9us of taps covers the xc1 DMA window) -> proj1 -> proj2 ->
            # conv1 -> proj3 -> conv2 -> ... (one-ahead so conv(s)'s first
            # tap never waits on proj(s)'s evictions). Dummy plugs absorb
            # residual DMA jitter during the ramp — a PE-idle gap there
            # resets the HAM activity window and costs ~6us.
            xp0 = proj(0, xcs.pop(0))
            w0 = ws.pop(0)
            xcs[2] = load_x(2)
            ws[2] = load_w(2)
            tick()
            warm(4)
            conv(0, xp0, w0)
            xcs[3] = load_x(3)
            ws[3] = load_w(3)
            xp1 = proj(1, xcs.pop(1))
            xcs[4] = load_x(4)
            ws[4] = load_w(4)
            tick()
            warm(4)
            prev = (1, xp1, ws.pop(1))
            for s in range(2, SPC):
                xp = proj(s, xcs.pop(s))
                if s + 3 < SPC:
                    xcs[s + 3] = load_x(s + 3)
                    ws[s + 3] = load_w(s + 3)
                if s == 2:
                    # xc2 lands near proj2's slot: absorb the jitter
                    tick()
                    warm(4)
                conv(*prev)
                prev = (s, xp, ws.pop(s))
            conv(*prev)

    nc.compile()
    return nc


def _prep(question_rep, lhs_rep, rhs_rep, proj_w, proj_b):
    """Host-side shard + layout prep (cheap reshapes/casts only)."""
    qr = np.ascontiguousarray(question_rep, dtype=np.float32)
    # conv weights: [B, o, i, kh, kw] -> [B, i, (kh kw), o] so each tap is a
    # ready lhsT [i, o] block and the per-sample weight DMA is contiguous
    qw = qr[:, :WDIM].reshape(B, DIM, DIM, K, K).transpose(0, 2, 3, 4, 1)
    qw = np.ascontiguousarray(qw).astype(BF_NP).reshape(B, DIM, KK * DIM)
    qb = np.ascontiguousarray(qr[:, WDIM:])             # [B, 128] fp32
    xl = np.asarray(lhs_rep, dtype=np.float32).reshape(B, DIM, 2, HALF)
    xr = np.asarray(rhs_rep, dtype=np.float32).reshape(B, DIM, 2, HALF)
    # pack [xl_h0 | xr_h0 | xl_h1 | xr_h1] -> [B, DIM, 4, HALF]
    xcm = np.stack([xl[:, :, 0], xr[:, :, 0], xl[:, :, 1], xr[:, :, 1]],
                   axis=2).astype(BF_NP)
    pwt = np.asarray(proj_w, dtype=np.float32).T.astype(BF_NP)  # [256, 128]
    pwm = np.ascontiguousarray(
        np.concatenate([pwt[:DIM], pwt[DIM:]], axis=1))  # [128, 256]
    pb = np.asarray(proj_b, dtype=np.float32).reshape(DIM, 1)

    in_maps = []
    for c in range(NCORES):
        sl = slice(c * SPC, (c + 1) * SPC)
        biam = np.ascontiguousarray(
            np.concatenate([qb[sl].T, pb], axis=1), dtype=np.float32)
        xcc = np.ascontiguousarray(xcm[sl])
        # pwx = [proj weights | this core's sample-0 xl_h0] (one ramp DMA)
        pwxm = np.ascontiguousarray(
            np.concatenate([pwm, xcc[0, :, 0, :]], axis=1))
        in_maps.append({
            "qw": np.ascontiguousarray(qw[sl]),
            "xc": xcc,
            "pwx": pwxm,
            "bia": biam,
        })
    return in_maps


def kernel(question_rep, lhs_rep, rhs_rep, proj_w, proj_b, _run_kwargs=None):
    if "nc" not in _BUILT:
        _BUILT["nc"] = build_nc()
    nc = _BUILT["nc"]
    in_maps = _prep(question_rep, lhs_rep, rhs_rep, proj_w, proj_b)
    res = run_bass_kernel_spmd(nc, in_maps, core_ids=list(range(NCORES)),
                               **(_run_kwargs or {}))
    out = np.concatenate(
        [np.asarray(res.results[c]["out"], dtype=np.float32)
         for c in range(NCORES)], axis=0)
    if _run_kwargs is not None:
        _BUILT["last_result"] = res
    return out.reshape(B, DIM, H, W)


if __name__ == "__main__":
    rng = np.random.default_rng(0)
    inputs = {
        "question_rep": rng.standard_normal((B, WDIM + DIM), dtype=np.float32) * 0.05,
        "lhs_rep": rng.standard_normal((B, DIM, H, W), dtype=np.float32),
        "rhs_rep": rng.standard_normal((B, DIM, H, W), dtype=np.float32),
        "proj_w": rng.standard_normal((DIM, 2 * DIM), dtype=np.float32),
        "proj_b": rng.standard_normal((DIM,), dtype=np.float32) * 0.01,
    }
    out = kernel(**inputs)
    print("ran, out shape:", out.shape)



# revision 48
# speedup vs baseline: 1.1024x; 1.1024x over previous
"""Dynamic per-sample CNN (nn_ConvFunc) Trainium2 Bass kernel.

Reference computation (per sample b):
  cnn_inp = proj_w @ cat(lhs, rhs) + proj_b          # 1x1 conv, [128, 32, 32]
  out     = conv3x3(cnn_inp, W_b) + bias_b           # W_b, bias_b unpacked from question_rep[b]

Sharding: pure data parallel, 8 samples per NeuronCore (batch 64 / 8 cores).

Per-core device kernel, per sample:
  - proj: per 512-col half of the 32x32 pixel space, 2 accumulating bf16
    matmuls (lhs-channels, rhs-channels) into one PSUM bank; ACT (h0) /
    DVE (h1) evict PSUM (+proj_b per-partition bias) into the interior of
    a zero-bordered [128,34,34] padded SBUF tile;
  - conv: per half, 9 taps of accumulating bf16 matmuls; rhs = shifted 16x32
    window of the padded tile (strided AP); DVE (h0) / ACT (h1) evict PSUM
    (+cnn bias) to bf16 SBUF, one sync-ring store DMA per sample (the last
    sample's second half is split into two PSUM banks so its evictions and
    stores overlap the final taps).

All matmul operands are bf16 (rounded on host): the PE streams 1 col/cycle
either way, but bf16 halves HBM traffic and triggers FWL so LDWEIGHTS hides
behind the previous matmul.

Scheduling notes (hard-won):
- The Tile scheduler is a readiness+priority simulator with an optimistic
  DMA model; left alone it reorders the Tensor queue and serializes the PE
  behind late DMAs. tick() stamps (tile_set_cur_wait) force the schedule.
- The PE must have NO idle gaps >~0.5us during the ramp or the HAM clock
  gate re-throttles to 1.2GHz (costs ~6us); dummy matmuls plug every
  DMA-jitter window.
- Tile has only 8 shared HWDGE DMA sem lanes: a 9th in-flight queue op
  stalls its ISSUING ENGINE until an older DMA completes. Loads are
  software-pipelined ~3 samples ahead, never all-upfront.
- Evictions (~700ns each, 4/sample, PSUM reads pinned to 1 elem/cycle) are
  split ACT/DVE so pairs run in parallel on different banks; store queue
  ops must NOT ride the scalar engine (they'd block ACT evictions).
- The last sample's second half accumulates into separate PSUM banks per
  quarter so the final evict+store chain is short and parallel.
"""

import numpy as np
import ml_dtypes

import concourse.bass as bass
import concourse.mybir as mybir
from concourse import bacc
from concourse.tile import TileContext
from concourse.bass_utils import run_bass_kernel_spmd

# Problem shapes (hardcoded per contract)
B = 64
DIM = 128
H = W = 32
K = 3
KK = K * K
HW = H * W             # 1024
WDIM = DIM * DIM * KK  # 147456
NCORES = 8
SPC = B // NCORES      # samples per core
HP, WP = H + 2, W + 2  # padded 34x34
HALF = HW // 2         # 512 columns per PSUM bank
QUART = HALF // 2
HROWS = H // 2         # 16 output rows per half
NXP = 4                # persistent padded tiles (cycled s % NXP)
N_WARM = 26            # dummy warmup matmuls (N=128 each)

FP = mybir.dt.float32
BF = mybir.dt.bfloat16
BF_NP = ml_dtypes.bfloat16

_BUILT = {}


def build_nc():
    # enable_partition_id=False: the kernel never branches on core id (inputs
    # are pre-sharded host-side), and the partition-id machinery costs ~1.7us
    # of per-engine TENSOR_LOADs in the program preamble.
    nc = bacc.Bacc("TRN2", target_bir_lowering=False, debug=False,
                   num_devices=NCORES, enable_partition_id=False)

    # xc packs [xl_h0 | xr_h0 | xl_h1 | xr_h1] per sample (one load DMA)
    xc = nc.declare_dram_parameter("xc", [SPC, DIM, 4, HALF], BF, isOutput=False)
    qw = nc.declare_dram_parameter("qw", [SPC, DIM, KK * DIM], BF, isOutput=False)
    # pwx packs [proj weights (256) | sample-0 xl_h0 (512)] so the entire
    # critical chain of the first real matmul lands with ONE DMA completion
    # (per-op completion latency ~2.3us dominates the ramp)
    pwx = nc.declare_dram_parameter("pwx", [DIM, 2 * DIM + HALF], BF,
                                    isOutput=False)
    bia = nc.declare_dram_parameter("bia", [DIM, SPC + 1], FP, isOutput=False)
    out = nc.declare_dram_parameter("out", [SPC, DIM, HW], BF, isOutput=True)

    with TileContext(nc) as tc:
        with (
            tc.tile_pool(name="const", bufs=1) as cpool,
            tc.tile_pool(name="wpool", bufs=6) as wpool,
            tc.tile_pool(name="xpool", bufs=6) as xpool,
            tc.tile_pool(name="opool", bufs=6) as opool,
            tc.tile_pool(name="pp_pool", bufs=4, space="PSUM") as pp_pool,
            tc.tile_pool(name="pc_pool", bufs=4, space="PSUM") as pc_pool,
        ):
            # --- warmup: short dummy matmuls keep the PE busy through the
            # DMA ramp so the HAM clock gate lifts before real work
            # memsets ride GpSimd: it exits the framework preamble earliest
            # and is otherwise idle, so the first warmup matmul issues sooner
            dummy = cpool.tile([DIM, DIM], BF)
            nc.gpsimd.memset(dummy[:], 0.0)
            pdt = pc_pool.tile([DIM, HALF], FP, tag="pc")

            def warm(n):
                for _ in range(n):
                    nc.tensor.matmul(pdt[:, 0:DIM], lhsT=dummy[:], rhs=dummy[:],
                                     start=True, stop=True)

            warm(N_WARM)

            # --- constants: pwx (proj weights + sample-0 xl_h0) FIRST on the
            # sync ring — one completion unblocks the first real matmul pair;
            # biases on scalar (needed only by the first eviction, later)
            pwx_sb = cpool.tile([DIM, 2 * DIM + HALF], BF)
            nc.sync.dma_start(out=pwx_sb[:], in_=pwx[:])
            bia_sb = cpool.tile([DIM, SPC + 1], FP)
            nc.scalar.dma_start(out=bia_sb[:], in_=bia[:])
            pw0 = pwx_sb[:, 0:DIM]
            pw1 = pwx_sb[:, DIM:2 * DIM]
            xl0_ap = pwx_sb[:, 2 * DIM:2 * DIM + HALF]

            def qb_ap(s):
                return bia_sb[:, s:s + 1]

            pb_ap = bia_sb[:, SPC:SPC + 1]

            # --- persistent padded tiles: borders zeroed once, interiors
            # rewritten per sample
            # xp borders on DVE (85-97ns each there vs 124-312 on GpSimd;
            # DVE is idle until the first eviction ~11us)
            xp_tiles = []
            for i in range(NXP):
                xp = cpool.tile([DIM, HP, WP], BF, tag=f"xp{i}")
                nc.vector.memset(xp[:, 0:1, :], 0.0)
                nc.vector.memset(xp[:, HP - 1:HP, :], 0.0)
                nc.vector.memset(xp[:, 1:HP - 1, 0:1], 0.0)
                nc.vector.memset(xp[:, 1:HP - 1, WP - 1:WP], 0.0)
                xp_tiles.append(xp)

            # ALL loads ride the sync ring in strict consumption order: the
            # ramp is bandwidth-bound (~150GB/s until ~11.5us, ~300+ after),
            # so sync-ring FIFO order IS the byte priority. The scalar ring
            # carries only bia and the per-sample stores, keeping the ACT
            # engine's instruction stream free of load queue ops.
            def load_x(s):
                xc_sb = xpool.tile([DIM, 4, HALF], BF, tag="xc")
                if s == 0:
                    # sample-0 xl_h0 rides inside pwx; xr_h0 then h1 (NOT
                    # finer: an 8th upfront queue op trips the 8-lane limit
                    # and stalls the whole ring's issue cadence)
                    nc.sync.dma_start(out=xc_sb[:, 1:2], in_=xc[s, :, 1:2])
                    nc.sync.dma_start(out=xc_sb[:, 2:4], in_=xc[s, :, 2:4])
                elif s <= 3:
                    # samples 1-3 sit on the ramp's byte wall: half-grain
                    # loads let proj(s)-h0 start ~0.85us before h1 lands
                    nc.sync.dma_start(out=xc_sb[:, 0:2], in_=xc[s, :, 0:2])
                    nc.sync.dma_start(out=xc_sb[:, 2:4], in_=xc[s, :, 2:4])
                else:
                    nc.sync.dma_start(out=xc_sb[:], in_=xc[s])
                return xc_sb

            def load_w(s):
                w_sb = wpool.tile([DIM, KK, DIM], BF, tag="w")
                if s == 0:
                    # tap-chunks so conv(0)'s first taps start on chunk 1
                    # (Tile's dependency tracking is range-precise)
                    for t0, t1 in ((0, 3), (3, KK)):
                        nc.sync.dma_start(out=w_sb[:, t0:t1, :],
                                          in_=qw[s, :, t0 * DIM:t1 * DIM])
                else:
                    # (SWDGE tried and rejected: Q7 descriptor-gen +
                    # packet round-robin starves the ramp's critical stream)
                    nc.sync.dma_start(out=w_sb[:], in_=qw[s])
                return w_sb

            # tick(): advance the Tile scheduler's logical clock. The
            # scheduler is a readiness+priority simulator whose DMA cost
            # model is optimistic — left alone it reorders the Tensor queue
            # (e.g. hoisting proj(s+1) above conv(s), which then serializes
            # the PE behind a late xc DMA and resets the HAM window).
            # Stamping each pipeline unit with an increasing wait_until makes
            # the compile-time order match THIS emission order; no runtime
            # waits are emitted (sems remain dependency-only).
            _clk = [0.0]

            def tick():
                _clk[0] += 0.05
                tc.tile_set_cur_wait(_clk[0])

            def proj(s, xc_sb):
                xp = xp_tiles[s % NXP]
                for h in range(2):
                    tick()
                    ppt = pp_pool.tile([DIM, HALF], FP, tag="pp")
                    xl_rhs = xl0_ap if (s == 0 and h == 0) else xc_sb[:, 2 * h]
                    nc.tensor.matmul(ppt[:], lhsT=pw0, rhs=xl_rhs,
                                     start=True, stop=False)
                    if s == 0 and h == 0:
                        # plug the xl->xr DMA-completion gap: PE-idle gaps
                        # during the ramp reset the HAM activity window
                        # (dummies write a different PSUM bank, which leaves
                        # ppt's accumulation group intact)
                        tick()
                        warm(4)
                        tick()
                    nc.tensor.matmul(ppt[:], lhsT=pw1, rhs=xc_sb[:, 2 * h + 1],
                                     start=False, stop=True)
                    # evictions are the serialization point (~750ns each, 4 per
                    # sample, PSUM reads pinned to 1 elem/cyc): split them over
                    # ACT (h0) and DVE (h1) so the two proj evicts run in
                    # parallel on different banks and conv(s)'s first taps
                    # (which need BOTH) unblock sooner
                    tick()
                    dst = xp[:, 1 + HROWS * h:1 + HROWS * (h + 1), 1:1 + W]
                    src = ppt[:].rearrange("p (a b) -> p a b", b=W)
                    if h == 0:
                        nc.scalar.activation(
                            dst, src, mybir.ActivationFunctionType.Identity,
                            bias=pb_ap)
                    else:
                        nc.vector.tensor_scalar_add(dst, src, pb_ap)
                    if s == 0 and h == 0:
                        # plug the xc0-h1 DMA wait (xl_h1 lands ~1.2us after
                        # xr at ramp bandwidth) so the HAM window survives
                        tick()
                        warm(12)
                return xp

            def conv(s, xp, w_sb):
                tick()
                o_sb = opool.tile([DIM, HW], BF, tag="o")
                last = s == SPC - 1
                if last:
                    # LAST sample, h1: accumulate the two quarters into TWO
                    # separate PSUM banks so quarter A's eviction + store
                    # overlap quarter B's taps, and the two evictions run on
                    # different banks (ACT || DVE) instead of serializing on
                    # one bank's single read port. Shrinks the kernel tail.
                    pct0 = pc_pool.tile([DIM, HALF], FP, tag="pc")
                    for t in range(KK):
                        kh, kw = divmod(t, K)
                        nc.tensor.matmul(
                            pct0[:],
                            lhsT=w_sb[:, t, :],
                            rhs=xp[:, kh:HROWS + kh, kw:kw + W],
                            start=(t == 0), stop=(t == KK - 1))
                    nc.vector.tensor_scalar_add(
                        o_sb[:, 0:HALF], pct0[:], qb_ap(s))
                    nc.sync.dma_start(out=out[s, :, 0:HALF],
                                      in_=o_sb[:, 0:HALF])
                    qrows = HROWS // 2  # 8 output rows per quarter
                    pcq = []
                    for q in range(2):
                        tick()
                        pq = pc_pool.tile([DIM, HALF], FP, tag="pc")
                        base = HROWS + qrows * q
                        for t in range(KK):
                            kh, kw = divmod(t, K)
                            nc.tensor.matmul(
                                pq[:, 0:QUART],
                                lhsT=w_sb[:, t, :],
                                rhs=xp[:, base + kh:base + qrows + kh,
                                       kw:kw + W],
                                start=(t == 0), stop=(t == KK - 1))
                        pcq.append(pq)
                        if q == 0:
                            # qA: DVE evict + sync store (overlaps qB's taps)
                            nc.vector.tensor_scalar_add(
                                o_sb[:, HALF:HALF + QUART], pq[:, 0:QUART],
                                qb_ap(s))
                            nc.sync.dma_start(
                                out=out[s, :, HALF:HALF + QUART],
                                in_=o_sb[:, HALF:HALF + QUART])
                    # final quarter: ACT evict (1.2GHz, fastest PSUM read)
                    # + store on the scalar ring, which has been empty since
                    # bia — no lane-reuse wait, fastest possible issue
                    nc.scalar.activation(
                        o_sb[:, HALF + QUART:HW], pcq[1][:, 0:QUART],
                        mybir.ActivationFunctionType.Identity,
                        bias=qb_ap(s))
                    nc.scalar.dma_start(out=out[s, :, HALF + QUART:HW],
                                        in_=o_sb[:, HALF + QUART:HW])
                    return
                for h in range(2):
                    tick()
                    pct = pc_pool.tile([DIM, HALF], FP, tag="pc")
                    for t in range(KK):
                        if s == 0 and h == 0 and t in (3, 6):
                            # absorb qw0 tap-chunk DMA jitter without a
                            # PE-idle gap (HAM window protection)
                            tick()
                            warm(2)
                            tick()
                        kh, kw = divmod(t, K)
                        nc.tensor.matmul(
                            pct[:],
                            lhsT=w_sb[:, t, :],
                            rhs=xp[:, HROWS * h + kh:HROWS * (h + 1) + kh,
                                   kw:kw + W],
                            start=(t == 0), stop=(t == KK - 1))
                    # conv evicts: h0 on DVE, h1 on ACT (proj uses the
                    # opposite pairing, balancing both engines)
                    dst = o_sb[:, h * HALF:(h + 1) * HALF]
                    if h == 0:
                        nc.vector.tensor_scalar_add(dst, pct[:], qb_ap(s))
                    else:
                        nc.scalar.activation(
                            dst, pct[:],
                            mybir.ActivationFunctionType.Identity,
                            bias=qb_ap(s))
                # store on the SYNC engine: a store queue op on the scalar
                # engine can block later ACT evictions for microseconds (it
                # waits on its DMA sem-lane's previous user + both evicts),
                # which starves conv(s+1) and resets the HAM window. The
                # sync engine has finished issuing all prefetched loads by
                # the time stores appear.
                nc.sync.dma_start(out=out[s], in_=o_sb[:])

            # Prefetch depth is bounded by Tile's 8 shared HWDGE sem lanes:
            # the 9th+ in-flight DMA queue op STALLS its issuing engine
            # waiting for an older DMA's completion. So only samples 0-1
            # load upfront (7 ops, unstamped = issue immediately); later
            # samples are emitted in-loop ~3 samples ahead — their stamps
            # give them issue slots well before consumption with only ~3
            # ops in flight.
            xcs = {0: load_x(0)}
            ws = {0: load_w(0)}
            xcs[1] = load_x(1)
            ws[1] = load_w(1)
            # Forced PE schedule (tick-stamped): proj0 -> conv0 (conv0's
            # 3.9us of taps covers the xc1 DMA window) -> proj1 -> proj2 ->
            # conv1 -> proj3 -> conv2 -> ... (one-ahead so conv(s)'s first
            # tap never waits on proj(s)'s evictions). Dummy plugs absorb
            # residual DMA jitter during the ramp — a PE-idle gap there
            # resets the HAM activity window and costs ~6us.
            xp0 = proj(0, xcs.pop(0))
            w0 = ws.pop(0)
            xcs[2] = load_x(2)
            ws[2] = load_w(2)
            tick()
            warm(4)
            conv(0, xp0, w0)
            xcs[3] = load_x(3)
            ws[3] = load_w(3)
            xp1 = proj(1, xcs.pop(1))
            xcs[4] = load_x(4)
            ws[4] = load_w(4)
            tick()
            warm(4)
            prev = (1, xp1, ws.pop(1))
            for s in range(2, SPC):
                xp = proj(s, xcs.pop(s))
                if s + 3 < SPC:
                    xcs[s + 3] = load_x(s + 3)
                    ws[s + 3] = load_w(s + 3)
                if s == 2:
                    # xc2 lands near proj2's slot: absorb the jitter
                    tick()
                    warm(4)
                conv(*prev)
                prev = (s, xp, ws.pop(s))
            conv(*prev)

    nc.compile()
    return nc


def _prep(question_rep, lhs_rep, rhs_rep, proj_w, proj_b):
    """Host-side shard + layout prep (cheap reshapes/casts only)."""
    qr = np.ascontiguousarray(question_rep, dtype=np.float32)
    # conv weights: [B, o, i, kh, kw] -> [B, i, (kh kw), o] so each tap is a
    # ready lhsT [i, o] block and the per-sample weight DMA is contiguous
    qw = qr[:, :WDIM].reshape(B, DIM, DIM, K, K).transpose(0, 2, 3, 4, 1)
    qw = np.ascontiguousarray(qw).astype(BF_NP).reshape(B, DIM, KK * DIM)
    qb = np.ascontiguousarray(qr[:, WDIM:])             # [B, 128] fp32
    xl = np.asarray(lhs_rep, dtype=np.float32).reshape(B, DIM, 2, HALF)
    xr = np.asarray(rhs_rep, dtype=np.float32).reshape(B, DIM, 2, HALF)
    # pack [xl_h0 | xr_h0 | xl_h1 | xr_h1] -> [B, DIM, 4, HALF]
    xcm = np.stack([xl[:, :, 0], xr[:, :, 0], xl[:, :, 1], xr[:, :, 1]],
                   axis=2).astype(BF_NP)
    pwt = np.asarray(proj_w, dtype=np.float32).T.astype(BF_NP)  # [256, 128]
    pwm = np.ascontiguousarray(
        np.concatenate([pwt[:DIM], pwt[DIM:]], axis=1))  # [128, 256]
    pb = np.asarray(proj_b, dtype=np.float32).reshape(DIM, 1)

    in_maps = []
    for c in range(NCORES):
        sl = slice(c * SPC, (c + 1) * SPC)
        biam = np.ascontiguousarray(
            np.concatenate([qb[sl].T, pb], axis=1), dtype=np.float32)
        xcc = np.ascontiguousarray(xcm[sl])
        # pwx = [proj weights | this core's sample-0 xl_h0] (one ramp DMA)
        pwxm = np.ascontiguousarray(
            np.concatenate([pwm, xcc[0, :, 0, :]], axis=1))
        in_maps.append({
            "qw": np.ascontiguousarray(qw[sl]),
            "xc": xcc,
            "pwx": pwxm,
            "bia": biam,
        })
    return in_maps


def kernel(question_rep, lhs_rep, rhs_rep, proj_w, proj_b, _run_kwargs=None):
    if "nc" not in _BUILT:
        _BUILT["nc"] = build_nc()
    nc = _BUILT["nc"]
    in_maps = _prep(question_rep, lhs_rep, rhs_rep, proj_w, proj_b)
    res = run_bass_kernel_spmd(nc, in_maps, core_ids=list(range(NCORES)),
                               **(_run_kwargs or {}))
    out = np.concatenate(
        [np.asarray(res.results[c]["out"], dtype=np.float32)
         for c in range(NCORES)], axis=0)
    if _run_kwargs is not None:
        _BUILT["last_result"] = res
    return out.reshape(B, DIM, H, W)


if __name__ == "__main__":
    rng = np.random.default_rng(0)
    inputs = {
        "question_rep": rng.standard_normal((B, WDIM + DIM), dtype=np.float32) * 0.05,
        "lhs_rep": rng.standard_normal((B, DIM, H, W), dtype=np.float32),
        "rhs_rep": rng.standard_normal((B, DIM, H, W), dtype=np.float32),
        "proj_w": rng.standard_normal((DIM, 2 * DIM), dtype=np.float32),
        "proj_b": rng.standard_normal((DIM,), dtype=np.float32) * 0.01,
    }
    out = kernel(**inputs)
    print("ran, out shape:", out.shape)



# revision 53
# speedup vs baseline: 1.1326x; 1.0274x over previous
"""Dynamic per-sample CNN (nn_ConvFunc) Trainium2 Bass kernel.

Reference computation (per sample b):
  cnn_inp = proj_w @ cat(lhs, rhs) + proj_b          # 1x1 conv, [128, 32, 32]
  out     = conv3x3(cnn_inp, W_b) + bias_b           # W_b, bias_b unpacked from question_rep[b]

Sharding: pure data parallel, 8 samples per NeuronCore (batch 64 / 8 cores).

Per-core device kernel, per sample:
  - proj: per 512-col half of the 32x32 pixel space, 2 accumulating bf16
    matmuls (lhs-channels, rhs-channels) into one PSUM bank; ACT (h0) /
    DVE (h1) evict PSUM (+proj_b per-partition bias) into the interior of
    a zero-bordered [128,34,34] padded SBUF tile;
  - conv: per half, 9 taps of accumulating bf16 matmuls; rhs = shifted 16x32
    window of the padded tile (strided AP); DVE (h0) / ACT (h1) evict PSUM
    (+cnn bias) to bf16 SBUF, one sync-ring store DMA per sample (the last
    sample's second half is split into two PSUM banks so its evictions and
    stores overlap the final taps).

All matmul operands are bf16 (rounded on host): the PE streams 1 col/cycle
either way, but bf16 halves HBM traffic and triggers FWL so LDWEIGHTS hides
behind the previous matmul.

Scheduling notes (hard-won):
- The Tile scheduler is a readiness+priority simulator with an optimistic
  DMA model; left alone it reorders the Tensor queue and serializes the PE
  behind late DMAs. tick() stamps (tile_set_cur_wait) force the schedule.
- The PE must have NO idle gaps >~0.5us during the ramp or the HAM clock
  gate re-throttles to 1.2GHz (costs ~6us); dummy matmuls plug every
  DMA-jitter window.
- Tile has only 8 shared HWDGE DMA sem lanes: a 9th in-flight queue op
  stalls its ISSUING ENGINE until an older DMA completes. Loads are
  software-pipelined ~3 samples ahead, never all-upfront.
- Evictions (~700ns each, 4/sample, PSUM reads pinned to 1 elem/cycle) are
  split ACT/DVE so pairs run in parallel on different banks; store queue
  ops must NOT ride the scalar engine (they'd block ACT evictions).
- The last sample's second half accumulates into separate PSUM banks per
  quarter so the final evict+store chain is short and parallel.
"""

import numpy as np
import ml_dtypes

import concourse.bass as bass
import concourse.mybir as mybir
from concourse import bacc
from concourse.tile import TileContext
from concourse.bass_utils import run_bass_kernel_spmd

# Problem shapes (hardcoded per contract)
B = 64
DIM = 128
H = W = 32
K = 3
KK = K * K
HW = H * W             # 1024
WDIM = DIM * DIM * KK  # 147456
NCORES = 8
SPC = B // NCORES      # samples per core
HP, WP = H + 2, W + 2  # padded 34x34
HALF = HW // 2         # 512 columns per PSUM bank
QUART = HALF // 2
HROWS = H // 2         # 16 output rows per half
NXP = 4                # persistent padded tiles (cycled s % NXP)
N_WARM = 26            # dummy warmup matmuls (N=128 each)

FP = mybir.dt.float32
BF = mybir.dt.bfloat16
BF_NP = ml_dtypes.bfloat16

_BUILT = {}


def build_nc():
    # enable_partition_id=False: the kernel never branches on core id (inputs
    # are pre-sharded host-side), and the partition-id machinery costs ~1.7us
    # of per-engine TENSOR_LOADs in the program preamble.
    nc = bacc.Bacc("TRN2", target_bir_lowering=False, debug=False,
                   num_devices=NCORES, enable_partition_id=False)

    # xc packs [xl_h0 | xr_h0 | xl_h1 | xr_h1] per sample (one load DMA)
    xc = nc.declare_dram_parameter("xc", [SPC, DIM, 4, HALF], BF, isOutput=False)
    qw = nc.declare_dram_parameter("qw", [SPC, DIM, KK * DIM], BF, isOutput=False)
    # pwx packs [proj weights (256) | sample-0 xl_h0 (512)] so the entire
    # critical chain of the first real matmul lands with ONE DMA completion
    # (per-op completion latency ~2.3us dominates the ramp)
    pwx = nc.declare_dram_parameter("pwx", [DIM, 2 * DIM + HALF], BF,
                                    isOutput=False)
    bia = nc.declare_dram_parameter("bia", [DIM, SPC + 1], FP, isOutput=False)
    out = nc.declare_dram_parameter("out", [SPC, DIM, HW], BF, isOutput=True)

    with TileContext(nc) as tc:
        with (
            tc.tile_pool(name="const", bufs=1) as cpool,
            tc.tile_pool(name="wpool", bufs=6) as wpool,
            tc.tile_pool(name="xpool", bufs=6) as xpool,
            tc.tile_pool(name="opool", bufs=6) as opool,
            tc.tile_pool(name="pp_pool", bufs=4, space="PSUM") as pp_pool,
            tc.tile_pool(name="pc_pool", bufs=4, space="PSUM") as pc_pool,
        ):
            # --- warmup: short dummy matmuls keep the PE busy through the
            # DMA ramp so the HAM clock gate lifts before real work
            # memsets ride GpSimd: it exits the framework preamble earliest
            # and is otherwise idle, so the first warmup matmul issues sooner
            dummy = cpool.tile([DIM, DIM], BF)
            nc.gpsimd.memset(dummy[:], 0.0)
            pdt = pc_pool.tile([DIM, HALF], FP, tag="pc")

            def warm(n):
                for _ in range(n):
                    nc.tensor.matmul(pdt[:, 0:DIM], lhsT=dummy[:], rhs=dummy[:],
                                     start=True, stop=True)

            warm(N_WARM)

            # --- constants: pwx (proj weights + sample-0 xl_h0) FIRST on the
            # sync ring — one completion unblocks the first real matmul pair;
            # biases on scalar (needed only by the first eviction, later)
            pwx_sb = cpool.tile([DIM, 2 * DIM + HALF], BF)
            nc.sync.dma_start(out=pwx_sb[:], in_=pwx[:])
            bia_sb = cpool.tile([DIM, SPC + 1], FP)
            nc.scalar.dma_start(out=bia_sb[:], in_=bia[:])
            pw0 = pwx_sb[:, 0:DIM]
            pw1 = pwx_sb[:, DIM:2 * DIM]
            xl0_ap = pwx_sb[:, 2 * DIM:2 * DIM + HALF]

            def qb_ap(s):
                return bia_sb[:, s:s + 1]

            pb_ap = bia_sb[:, SPC:SPC + 1]

            # --- persistent padded tiles: borders zeroed once, interiors
            # rewritten per sample
            # xp borders on DVE (85-97ns each there vs 124-312 on GpSimd;
            # DVE is idle until the first eviction ~11us)
            xp_tiles = []
            for i in range(NXP):
                xp = cpool.tile([DIM, HP, WP], BF, tag=f"xp{i}")
                nc.vector.memset(xp[:, 0:1, :], 0.0)
                nc.vector.memset(xp[:, HP - 1:HP, :], 0.0)
                nc.vector.memset(xp[:, 1:HP - 1, 0:1], 0.0)
                nc.vector.memset(xp[:, 1:HP - 1, WP - 1:WP], 0.0)
                xp_tiles.append(xp)

            # ALL loads ride the sync ring in strict consumption order: the
            # ramp is bandwidth-bound (~150GB/s until ~11.5us, ~300+ after),
            # so sync-ring FIFO order IS the byte priority. The scalar ring
            # carries only bia and the per-sample stores, keeping the ACT
            # engine's instruction stream free of load queue ops.
            def load_x(s):
                xc_sb = xpool.tile([DIM, 4, HALF], BF, tag="xc")
                if s == 0:
                    # sample-0 xl_h0 rides inside pwx; xr_h0 then h1 (NOT
                    # finer: an 8th upfront queue op trips the 8-lane limit
                    # and stalls the whole ring's issue cadence)
                    nc.sync.dma_start(out=xc_sb[:, 1:2], in_=xc[s, :, 1:2])
                    nc.sync.dma_start(out=xc_sb[:, 2:4], in_=xc[s, :, 2:4])
                elif s <= 3:
                    # samples 1-3 sit on the ramp's byte wall: half-grain
                    # loads let proj(s)-h0 start ~0.85us before h1 lands
                    nc.sync.dma_start(out=xc_sb[:, 0:2], in_=xc[s, :, 0:2])
                    nc.sync.dma_start(out=xc_sb[:, 2:4], in_=xc[s, :, 2:4])
                else:
                    nc.sync.dma_start(out=xc_sb[:], in_=xc[s])
                return xc_sb

            def load_w(s):
                w_sb = wpool.tile([DIM, KK, DIM], BF, tag="w")
                if s == 0:
                    # tap-chunks so conv(0)'s first taps start on chunk 1
                    # (Tile's dependency tracking is range-precise)
                    for t0, t1 in ((0, 3), (3, KK)):
                        nc.sync.dma_start(out=w_sb[:, t0:t1, :],
                                          in_=qw[s, :, t0 * DIM:t1 * DIM])
                else:
                    # (SWDGE tried and rejected: Q7 descriptor-gen +
                    # packet round-robin starves the ramp's critical stream)
                    nc.sync.dma_start(out=w_sb[:], in_=qw[s])
                return w_sb

            # tick(): advance the Tile scheduler's logical clock. The
            # scheduler is a readiness+priority simulator whose DMA cost
            # model is optimistic — left alone it reorders the Tensor queue
            # (e.g. hoisting proj(s+1) above conv(s), which then serializes
            # the PE behind a late xc DMA and resets the HAM window).
            # Stamping each pipeline unit with an increasing wait_until makes
            # the compile-time order match THIS emission order; no runtime
            # waits are emitted (sems remain dependency-only).
            _clk = [0.0]

            def tick():
                _clk[0] += 0.05
                tc.tile_set_cur_wait(_clk[0])

            def proj(s, xc_sb):
                xp = xp_tiles[s % NXP]
                for h in range(2):
                    tick()
                    ppt = pp_pool.tile([DIM, HALF], FP, tag="pp")
                    xl_rhs = xl0_ap if (s == 0 and h == 0) else xc_sb[:, 2 * h]
                    nc.tensor.matmul(ppt[:], lhsT=pw0, rhs=xl_rhs,
                                     start=True, stop=False)
                    if s == 0 and h == 0:
                        # plug the xl->xr DMA-completion gap: PE-idle gaps
                        # during the ramp reset the HAM activity window
                        # (dummies write a different PSUM bank, which leaves
                        # ppt's accumulation group intact)
                        tick()
                        warm(4)
                        tick()
                    nc.tensor.matmul(ppt[:], lhsT=pw1, rhs=xc_sb[:, 2 * h + 1],
                                     start=False, stop=True)
                    # evictions are the serialization point (~750ns each, 4 per
                    # sample, PSUM reads pinned to 1 elem/cyc): split them over
                    # ACT (h0) and DVE (h1) so the two proj evicts run in
                    # parallel on different banks and conv(s)'s first taps
                    # (which need BOTH) unblock sooner
                    tick()
                    dst = xp[:, 1 + HROWS * h:1 + HROWS * (h + 1), 1:1 + W]
                    src = ppt[:].rearrange("p (a b) -> p a b", b=W)
                    if h == 0:
                        nc.scalar.activation(
                            dst, src, mybir.ActivationFunctionType.Identity,
                            bias=pb_ap)
                    else:
                        nc.vector.tensor_scalar_add(dst, src, pb_ap)
                    if s == 0 and h == 0:
                        # plug the xc0-h1 DMA wait (xl_h1 lands ~1.2us after
                        # xr at ramp bandwidth) so the HAM window survives
                        tick()
                        warm(12)
                return xp

            def conv(s, xp, w_sb):
                tick()
                o_sb = opool.tile([DIM, HW], BF, tag="o")
                last = s == SPC - 1
                if last:
                    # LAST sample, h1: accumulate the two quarters into TWO
                    # separate PSUM banks so quarter A's eviction + store
                    # overlap quarter B's taps, and the two evictions run on
                    # different banks (ACT || DVE) instead of serializing on
                    # one bank's single read port. Shrinks the kernel tail.
                    pct0 = pc_pool.tile([DIM, HALF], FP, tag="pc")
                    for t in range(KK):
                        kh, kw = divmod(t, K)
                        nc.tensor.matmul(
                            pct0[:],
                            lhsT=w_sb[:, t, :],
                            rhs=xp[:, kh:HROWS + kh, kw:kw + W],
                            start=(t == 0), stop=(t == KK - 1))
                    nc.vector.tensor_scalar_add(
                        o_sb[:, 0:HALF], pct0[:], qb_ap(s))
                    nc.sync.dma_start(out=out[s, :, 0:HALF],
                                      in_=o_sb[:, 0:HALF])
                    qrows = HROWS // 2  # 8 output rows per quarter
                    pcq = []
                    for q in range(2):
                        tick()
                        pq = pc_pool.tile([DIM, HALF], FP, tag="pc")
                        base = HROWS + qrows * q
                        for t in range(KK):
                            kh, kw = divmod(t, K)
                            nc.tensor.matmul(
                                pq[:, 0:QUART],
                                lhsT=w_sb[:, t, :],
                                rhs=xp[:, base + kh:base + qrows + kh,
                                       kw:kw + W],
                                start=(t == 0), stop=(t == KK - 1))
                        pcq.append(pq)
                        if q == 0:
                            # qA: DVE evict + sync store (overlaps qB's taps)
                            nc.vector.tensor_scalar_add(
                                o_sb[:, HALF:HALF + QUART], pq[:, 0:QUART],
                                qb_ap(s))
                            nc.sync.dma_start(
                                out=out[s, :, HALF:HALF + QUART],
                                in_=o_sb[:, HALF:HALF + QUART])
                    # final quarter: ACT evict (1.2GHz, fastest PSUM read)
                    # + store on the scalar ring, which has been empty since
                    # bia — no lane-reuse wait, fastest possible issue
                    nc.scalar.activation(
                        o_sb[:, HALF + QUART:HW], pcq[1][:, 0:QUART],
                        mybir.ActivationFunctionType.Identity,
                        bias=qb_ap(s))
                    nc.scalar.dma_start(out=out[s, :, HALF + QUART:HW],
                                        in_=o_sb[:, HALF + QUART:HW])
                    return
                for h in range(2):
                    tick()
                    pct = pc_pool.tile([DIM, HALF], FP, tag="pc")
                    for t in range(KK):
                        kh, kw = divmod(t, K)
                        nc.tensor.matmul(
                            pct[:],
                            lhsT=w_sb[:, t, :],
                            rhs=xp[:, HROWS * h + kh:HROWS * (h + 1) + kh,
                                   kw:kw + W],
                            start=(t == 0), stop=(t == KK - 1))
                    # conv evicts: h0 on DVE, h1 on ACT (proj uses the
                    # opposite pairing, balancing both engines)
                    dst = o_sb[:, h * HALF:(h + 1) * HALF]
                    if h == 0:
                        nc.vector.tensor_scalar_add(dst, pct[:], qb_ap(s))
                    else:
                        nc.scalar.activation(
                            dst, pct[:],
                            mybir.ActivationFunctionType.Identity,
                            bias=qb_ap(s))
                # store on the SYNC engine: a store queue op on the scalar
                # engine can block later ACT evictions for microseconds (it
                # waits on its DMA sem-lane's previous user + both evicts),
                # which starves conv(s+1) and resets the HAM window. The
                # sync engine has finished issuing all prefetched loads by
                # the time stores appear.
                nc.sync.dma_start(out=out[s], in_=o_sb[:])

            # Prefetch depth is bounded by Tile's 8 shared HWDGE sem lanes:
            # the 9th+ in-flight DMA queue op STALLS its issuing engine
            # waiting for an older DMA's completion. So only samples 0-1
            # load upfront (7 ops, unstamped = issue immediately); later
            # samples are emitted in-loop ~3 samples ahead — their stamps
            # give them issue slots well before consumption with only ~3
            # ops in flight.
            xcs = {0: load_x(0)}
            ws = {0: load_w(0)}
            xcs[1] = load_x(1)
            # Forced PE schedule (tick-stamped): proj0 -> conv0 (conv0's
            # 3.9us of taps covers the xc1 DMA window) -> proj1 -> proj2 ->
            # conv1 -> proj3 -> conv2 -> ... (one-ahead so conv(s)'s first
            # tap never waits on proj(s)'s evictions). Dummy plugs absorb
            # residual DMA jitter during the ramp — a PE-idle gap there
            # resets the HAM activity window and costs ~6us.
            # Post-lift dummy plugs are PURE COST: once HAM is warm
            # (~10.5us), an idle gap <3.4us costs only the gap, while a
            # scheduled dummy always costs its duration even when data
            # arrived early. Only ramp plugs (inside proj0/conv0) remain.
            xp0 = proj(0, xcs.pop(0))
            w0 = ws.pop(0)
            # xc2 rides the ring BEFORE qw1: proj2 needs xc2 ~2us before
            # conv1 needs qw1
            xcs[2] = load_x(2)
            ws[1] = load_w(1)
            ws[2] = load_w(2)
            tick()
            conv(0, xp0, w0)
            xcs[3] = load_x(3)
            ws[3] = load_w(3)
            xp1 = proj(1, xcs.pop(1))
            xcs[4] = load_x(4)
            ws[4] = load_w(4)
            tick()
            prev = (1, xp1, ws.pop(1))
            for s in range(2, SPC):
                xp = proj(s, xcs.pop(s))
                if s + 3 < SPC:
                    xcs[s + 3] = load_x(s + 3)
                    ws[s + 3] = load_w(s + 3)
                conv(*prev)
                prev = (s, xp, ws.pop(s))
            conv(*prev)

    nc.compile()
    return nc


def _prep(question_rep, lhs_rep, rhs_rep, proj_w, proj_b):
    """Host-side shard + layout prep (cheap reshapes/casts only)."""
    qr = np.ascontiguousarray(question_rep, dtype=np.float32)
    # conv weights: [B, o, i, kh, kw] -> [B, i, (kh kw), o] so each tap is a
    # ready lhsT [i, o] block and the per-sample weight DMA is contiguous
    qw = qr[:, :WDIM].reshape(B, DIM, DIM, K, K).transpose(0, 2, 3, 4, 1)
    qw = np.ascontiguousarray(qw).astype(BF_NP).reshape(B, DIM, KK * DIM)
    qb = np.ascontiguousarray(qr[:, WDIM:])             # [B, 128] fp32
    xl = np.asarray(lhs_rep, dtype=np.float32).reshape(B, DIM, 2, HALF)
    xr = np.asarray(rhs_rep, dtype=np.float32).reshape(B, DIM, 2, HALF)
    # pack [xl_h0 | xr_h0 | xl_h1 | xr_h1] -> [B, DIM, 4, HALF]
    xcm = np.stack([xl[:, :, 0], xr[:, :, 0], xl[:, :, 1], xr[:, :, 1]],
                   axis=2).astype(BF_NP)
    pwt = np.asarray(proj_w, dtype=np.float32).T.astype(BF_NP)  # [256, 128]
    pwm = np.ascontiguousarray(
        np.concatenate([pwt[:DIM], pwt[DIM:]], axis=1))  # [128, 256]
    pb = np.asarray(proj_b, dtype=np.float32).reshape(DIM, 1)

    in_maps = []
    for c in range(NCORES):
        sl = slice(c * SPC, (c + 1) * SPC)
        biam = np.ascontiguousarray(
            np.concatenate([qb[sl].T, pb], axis=1), dtype=np.float32)
        xcc = np.ascontiguousarray(xcm[sl])
        # pwx = [proj weights | this core's sample-0 xl_h0] (one ramp DMA)
        pwxm = np.ascontiguousarray(
            np.concatenate([pwm, xcc[0, :, 0, :]], axis=1))
        in_maps.append({
            "qw": np.ascontiguousarray(qw[sl]),
            "xc": xcc,
            "pwx": pwxm,
            "bia": biam,
        })
    return in_maps


def kernel(question_rep, lhs_rep, rhs_rep, proj_w, proj_b, _run_kwargs=None):
    if "nc" not in _BUILT:
        _BUILT["nc"] = build_nc()
    nc = _BUILT["nc"]
    in_maps = _prep(question_rep, lhs_rep, rhs_rep, proj_w, proj_b)
    res = run_bass_kernel_spmd(nc, in_maps, core_ids=list(range(NCORES)),
                               **(_run_kwargs or {}))
    out = np.concatenate(
        [np.asarray(res.results[c]["out"], dtype=np.float32)
         for c in range(NCORES)], axis=0)
    if _run_kwargs is not None:
        _BUILT["last_result"] = res
    return out.reshape(B, DIM, H, W)


if __name__ == "__main__":
    rng = np.random.default_rng(0)
    inputs = {
        "question_rep": rng.standard_normal((B, WDIM + DIM), dtype=np.float32) * 0.05,
        "lhs_rep": rng.standard_normal((B, DIM, H, W), dtype=np.float32),
        "rhs_rep": rng.standard_normal((B, DIM, H, W), dtype=np.float32),
        "proj_w": rng.standard_normal((DIM, 2 * DIM), dtype=np.float32),
        "proj_b": rng.standard_normal((DIM,), dtype=np.float32) * 0.01,
    }
    out = kernel(**inputs)
    print("ran, out shape:", out.shape)

